# revision 11
# baseline (speedup 1.0000x reference)
"""BiLSTM + CRF (NCRF++ style) Trainium2 Bass kernel.

Sharding: direction-split + batch-split across 8 NeuronCores.
  cores 0-3: forward LSTM, batch quarters 0-3 (8 seqs each)
  cores 4-7: backward LSTM (time-reversed input data), batch quarters 0-3
All cores run the SAME NEFF; only input data differs (SPMD).

Per-core device work:
  1. input projection xW = x @ Wih'.T + b'  (batched over all tokens)
  2. 256-step LSTM recurrence (weights-stationary fp32 matmuls,
     gate rows on the 128 partitions)
  3. partial logit projection P = h_all @ (0.5*W_tag_half).T

Gate rows are host-reordered to [i,f,o,g] so one Sigmoid ACT op covers
i,f,o (contiguous free cols) and one Tanh covers g. The i/f/o/g biases
fold into the input projection via a constant-1 feature column.

Host does: embedding gather, data packing, CRF-NLL + Viterbi tail
(0.02% of FLOPs) in numpy.
"""

import numpy as np


def _ensure_path():
    import sys
    for p in ("/opt/trn_rl_repo",):
        if p not in sys.path:
            sys.path.insert(0, p)


_ensure_path()

try:
    from antenv import axon_hooks as _ah  # noqa: F401
except Exception:
    import sys as _s
    import types as _t
    _m = _t.ModuleType("antenv.axon_hooks")
    _m.get_axon_ntff_profile_hook = lambda: None
    _s.modules["antenv.axon_hooks"] = _m

import bass_rust  # noqa: E402
import concourse.bass as bass  # noqa: E402
import concourse.mybir as mybir  # noqa: E402
from concourse.tile import TileContext  # noqa: E402
from concourse.bass_utils import run_bass_kernel_spmd  # noqa: E402

B, S, E, H = 32, 256, 300, 256
T = 19
START, STOP = 17, 18
EP = 384           # E padded to 3*128
BL = B // 4        # sequences per core (8)
KC_X = EP // 128   # K chunks for input projection (3)
KC_H = H // 128    # K chunks for Whh (2)
MC = (4 * H) // 128  # M chunks of gate dim (8)
NTOK = S * BL      # tokens per core (2048)
NBLK = NTOK // 512  # token blocks for batched matmuls (4)

F32 = mybir.dt.float32
ADD = mybir.AluOpType.add
MULT = mybir.AluOpType.mult
TANH = mybir.ActivationFunctionType.Tanh
SIGM = mybir.ActivationFunctionType.Sigmoid

LAST_RESULT = None
LAST_IN_MAPS = None
_NC_CACHE = {}


def _build_bass():
    nc = bass.Bass()
    xt_d = nc.dram_tensor("xt", [128, KC_X, NTOK], F32, kind="ExternalInput")
    wih_d = nc.dram_tensor("wih", [128, KC_X, MC, 128], F32, kind="ExternalInput")
    whh_d = nc.dram_tensor("whh", [128, KC_H, MC, 128], F32, kind="ExternalInput")
    wtg_d = nc.dram_tensor("wtg", [128, KC_H, T], F32, kind="ExternalInput")
    p_d = nc.dram_tensor("p", [T, NTOK], F32, kind="ExternalOutput")

    with TileContext(nc) as tc:
        with tc.tile_pool(name="const", bufs=1) as cpool, \
             tc.tile_pool(name="xw", bufs=1) as xwpool, \
             tc.tile_pool(name="hall", bufs=1) as hpool, \
             tc.tile_pool(name="work", bufs=3) as wpool, \
             tc.tile_pool(name="state", bufs=2) as spool, \
             tc.tile_pool(name="psx", bufs=4, space="PSUM") as psxpool, \
             tc.tile_pool(name="psg", bufs=3, space="PSUM") as psgpool, \
             tc.tile_pool(name="psl", bufs=1, space="PSUM") as pslpool:

            xt = cpool.tile([128, KC_X, NTOK], F32)
            wih = cpool.tile([128, KC_X, MC, 128], F32)
            whh = cpool.tile([128, KC_H, MC, 128], F32)
            wtg = cpool.tile([128, KC_H, T], F32)
            # Funnel all PE-read tiles through DVE copies: a self-loading
            # fp32 Matmult has very few semaphore-wait slots, so all its
            # upstream deps must collapse onto one (DVE) semaphore.
            with tc.tile_pool(name="stage", bufs=1) as stpool:
                xt_s = stpool.tile([128, KC_X, NTOK], F32)
                wih_s = stpool.tile([128, KC_X, MC, 128], F32)
                whh_s = stpool.tile([128, KC_H, MC, 128], F32)
                wtg_s = stpool.tile([128, KC_H, T], F32)
                nc.sync.dma_start(wih_s[:, :, :, :], wih_d[:, :, :, :])
                nc.sync.dma_start(whh_s[:, :, :, :], whh_d[:, :, :, :])
                nc.sync.dma_start(wtg_s[:, :, :], wtg_d[:, :, :])
                for k in range(KC_X):
                    nc.sync.dma_start(xt_s[:, k, :], xt_d[:, k, :])
                nc.vector.tensor_copy(wih[:, :, :, :], wih_s[:, :, :, :])
                nc.vector.tensor_copy(whh[:, :, :, :], whh_s[:, :, :, :])
                nc.vector.tensor_copy(wtg[:, :, :], wtg_s[:, :, :])
                for k in range(KC_X):
                    nc.vector.tensor_copy(xt[:, k, :], xt_s[:, k, :])

            # xW[p, t, m, b]: pre-gate input contribution per token
            xw = xwpool.tile([128, S, MC, BL], F32)
            # h_all[p, k, t, b]: hidden states (stored as 2h)
            hall = hpool.tile([128, KC_H, S, BL], F32)

            # ---- phase 1: input projection ----
            for nb in range(NBLK):
                for m in range(MC):
                    ps = psxpool.tile([128, 64, BL], F32)
                    for k in range(KC_X):
                        nc.tensor.matmul(
                            ps[:, :, :],
                            wih[:, k, m, :],
                            xt[:, k, nb * 512:(nb + 1) * 512],
                            start=(k == 0), stop=(k == KC_X - 1),
                        )
                    nc.vector.tensor_copy(
                        xw[:, nb * 64:(nb + 1) * 64, m, :], ps[:, :, :])

            # ---- phase 2: LSTM recurrence ----
            c_prev = spool.tile([128, KC_H, BL], F32, tag="c")
            nc.vector.memset(c_prev[:, :, :], 0.0)
            for t in range(S):
                if t > 0:
                    g = psgpool.tile([128, MC, BL], F32)
                    for m in range(MC):
                        for k in range(KC_H):
                            nc.tensor.matmul(
                                g[:, m, :],
                                whh[:, k, m, :],
                                hall[:, k, t - 1, :],
                                start=(k == 0), stop=(k == KC_H - 1),
                            )
                    pre = wpool.tile([128, MC, BL], F32, tag="pre")
                    nc.vector.tensor_tensor(
                        pre[:, :, :], g[:, :, :], xw[:, t, :, :], ADD)
                    pre_ap = pre[:, :, :]
                else:
                    pre_ap = xw[:, 0, :, :]
                # gate chunks (host reorder): m 0-1=i, 2-3=f, 4-5=o, 6-7=g
                sg = wpool.tile([128, 6, BL], F32, tag="sg")
                nc.scalar.activation(sg[:, :, :], pre_ap[:, 0:6, :], SIGM)
                tg = wpool.tile([128, KC_H, BL], F32, tag="tg")
                nc.scalar.activation(tg[:, :, :], pre_ap[:, 6:8, :], TANH)
                t1 = wpool.tile([128, KC_H, BL], F32, tag="t1")
                t2 = wpool.tile([128, KC_H, BL], F32, tag="t2")
                c_new = spool.tile([128, KC_H, BL], F32, tag="c")
                nc.vector.tensor_tensor(
                    t1[:, :, :], sg[:, 2:4, :], c_prev[:, :, :], MULT)
                nc.vector.tensor_tensor(
                    t2[:, :, :], sg[:, 0:2, :], tg[:, :, :], MULT)
                nc.vector.tensor_tensor(
                    c_new[:, :, :], t1[:, :, :], t2[:, :, :], ADD)
                tct = wpool.tile([128, KC_H, BL], F32, tag="tct")
                nc.scalar.activation(tct[:, :, :], c_new[:, :, :], TANH)
                nc.vector.tensor_tensor(
                    hall[:, :, t, :], sg[:, 4:6, :], tct[:, :, :], MULT)
                c_prev = c_new

            # ---- phase 3: partial logits ----
            p_sb = cpool.tile([T, NTOK], F32)
            for nb in range(NBLK):
                pl = pslpool.tile([T, 512], F32)
                for k in range(KC_H):
                    nc.tensor.matmul(
                        pl[:, :],
                        wtg[:, k, :],
                        hall[:, k, nb * 64:(nb + 1) * 64, :],
                        start=(k == 0), stop=(k == KC_H - 1),
                    )
                nc.vector.tensor_copy(p_sb[:, nb * 512:(nb + 1) * 512], pl[:, :])
            nc.sync.dma_start(p_d[:, :], p_sb[:, :])
    _split_multiwaits(nc)
    return nc


def _split_multiwaits(nc):
    """This toolchain's walrus accepts only ONE sync-wait per compute
    instruction. Split extra waits onto engine NoOps inserted just
    before the offender (same engine, same position => semantics kept)."""
    wid = 0
    for f in nc.m.functions:
        for bb in f.blocks:
            out = []
            changed = False
            for inst in bb.instructions:
                si = inst.sync_info
                if si is not None and si.on_wait is not None and len(si.on_wait) > 1:
                    waits = list(si.on_wait)
                    for w in waits[:-1]:
                        wid += 1
                        out.append(bass_rust.InstNoOp(
                            name=f"WS-{wid}", engine=inst.engine,
                            sync_info=mybir.SyncInfo(on_wait=[w], on_update=[])))
                    inst.sync_info = mybir.SyncInfo(
                        on_wait=[waits[-1]],
                        on_update=list(si.on_update or []))
                    changed = True
                out.append(inst)
            if changed:
                bb.instructions = out


def _get_nc():
    if "nc" not in _NC_CACHE:
        _NC_CACHE["nc"] = _build_bass()
    return _NC_CACHE["nc"]


def _pack_weights(Wih, Whh, bih, bhh, Wtag_half):
    """Reorder gate rows i,f,g,o -> i,f,o,g; scale i,f,o rows by 0.5
    (sigmoid-via-tanh); scale Whh by extra 0.5 (input is 2h)."""
    perm = np.concatenate(
        [np.arange(0, 512), np.arange(768, 1024), np.arange(512, 768)])
    Wih2 = Wih[perm].astype(np.float32)
    Whh2 = Whh[perm].astype(np.float32)
    b2 = (bih + bhh)[perm].astype(np.float32)

    Wp = np.zeros((4 * H, EP), np.float32)
    Wp[:, :E] = Wih2
    Wp[:, E] = b2  # bias folded in via constant-1 feature row E
    wih_arr = np.ascontiguousarray(
        Wp.T.reshape(KC_X, 128, MC, 128).transpose(1, 0, 2, 3))
    whh_arr = np.ascontiguousarray(
        Whh2.T.reshape(KC_H, 128, MC, 128).transpose(1, 0, 2, 3))
    Wt = Wtag_half.astype(np.float32)  # [T, H]
    wtg_arr = np.ascontiguousarray(
        Wt.T.reshape(KC_H, 128, T).transpose(1, 0, 2))
    return wih_arr, whh_arr, wtg_arr


def _pack_x(xq):
    """xq [BL, S, E] -> [128, KC_X, NTOK] with token j = t*BL + b."""
    xp = np.zeros((BL, S, EP), np.float32)
    xp[:, :, :E] = xq
    xp[:, :, E] = 1.0
    arr = xp.transpose(2, 1, 0).reshape(KC_X, 128, NTOK)
    return np.ascontiguousarray(arr.transpose(1, 0, 2))


def _crf_nll_np(emit, mask, labels, trans):
    emit = emit.astype(np.float64)
    trans = trans.astype(np.float64)
    mf = mask.astype(np.float64)
    b = emit.shape[0]
    prev = np.concatenate(
        [np.full((b, 1), START, labels.dtype), labels[:, :-1]], axis=1)
    trans_sc = trans[prev, labels]
    emit_sc = np.take_along_axis(emit, labels[..., None], axis=-1)[..., 0]
    gold = np.sum((trans_sc + emit_sc) * mf, axis=1)
    lengths = mask.sum(axis=1).astype(np.int64)
    last_tag = np.take_along_axis(labels, (lengths - 1)[:, None], axis=1)[:, 0]
    gold = gold + trans[last_tag, STOP]
    alpha = emit[:, 0] + trans[START][None, :]
    for t in range(1, emit.shape[1]):
        cand = alpha[:, :, None] + trans[None]        # [B, prev, cur]
        mx = cand.max(axis=1)
        new = mx + np.log(np.exp(cand - mx[:, None, :]).sum(axis=1)) + emit[:, t]
        alpha = np.where(mask[:, t][:, None] > 0, new, alpha)
    final = alpha + trans[:, STOP][None, :]
    mx = final.max(axis=1)
    logZ = mx + np.log(np.exp(final - mx[:, None]).sum(axis=1))
    return np.sum(logZ - gold)


def _viterbi_np(emit, mask, trans):
    emit = emit.astype(np.float32)
    b, s, t = emit.shape
    delta = emit[:, 0] + trans[START][None, :]
    idT = np.arange(t, dtype=np.int32)[None, :]
    ptrs = []
    for i in range(1, s):
        cand = delta[:, :, None] + trans[None]        # [B, prev, cur]
        best_prev = cand.argmax(axis=1).astype(np.int32)
        new = cand.max(axis=1) + emit[:, i]
        m = mask[:, i][:, None] > 0
        delta = np.where(m, new, delta)
        ptrs.append(np.where(m, best_prev, idT))
    final = delta + trans[:, STOP][None, :]
    best_last = final.argmax(axis=1).astype(np.int32)
    bidx = np.arange(b)
    ys = np.empty((b, s - 1), np.int32)
    carry = best_last
    for i in range(s - 2, -1, -1):
        ys[:, i] = carry
        carry = ptrs[i][bidx, carry]
    tags = np.concatenate([carry[:, None], ys], axis=1)
    return tags * (mask > 0)


def bench_device(in_maps, iters=10):
    """Time the NEFF execution via PJRT with device-resident inputs.

    Mirrors bass2jax.run_bass_via_pjrt's shard_map path but without
    donation, inputs device_put once, executable reused across iters.
    Returns per-iteration wall seconds (min over iters), which bounds
    the HW exec time from above (adds one dispatch round-trip).
    """
    import time
    import jax
    from jax.experimental.shard_map import shard_map
    from jax.sharding import Mesh, NamedSharding, PartitionSpec
    from concourse import bass2jax as b2j

    nc = _get_nc()
    b2j.install_neuronx_cc_hook()
    n_cores = len(in_maps)
    partition_name = (nc.partition_id_tensor.name
                      if nc.partition_id_tensor else None)

    in_names, out_names, out_avals, zero_outs = [], [], [], []
    for alloc in nc.m.functions[0].allocations:
        if not isinstance(alloc, mybir.MemoryLocationSet):
            continue
        name = alloc.memorylocations[0].name
        if alloc.kind == "ExternalInput":
            if name != partition_name:
                in_names.append(name)
        elif alloc.kind == "ExternalOutput":
            shape = tuple(alloc.tensor_shape)
            dtype = mybir.dt.np(alloc.dtype)
            out_names.append(name)
            out_avals.append(jax.core.ShapedArray(shape, dtype))
            zero_outs.append(np.zeros(shape, dtype))
    n_params = len(in_names)
    all_names = in_names + out_names
    if partition_name is not None:
        all_names = all_names + [partition_name]

    def _body(*args):
        operands = list(args)
        if partition_name is not None:
            operands.append(b2j.partition_id_tensor())
        outs = b2j._bass_exec_p.bind(
            *operands,
            out_avals=tuple(out_avals),
            in_names=tuple(all_names),
            out_names=tuple(out_names),
            lowering_input_output_aliases=(),
            sim_require_finite=True,
            sim_require_nnan=True,
            nc=nc,
        )
        return tuple(outs)

    devices = jax.devices()[:n_cores]
    mesh = Mesh(np.asarray(devices), ("core",))
    spec = PartitionSpec("core")
    in_specs = (spec,) * (n_params + len(out_names))
    out_specs = (spec,) * len(out_names)
    f = jax.jit(shard_map(_body, mesh=mesh, in_specs=in_specs,
                          out_specs=out_specs, check_rep=False),
                keep_unused=True)
    sh = NamedSharding(mesh, spec)
    concat_in = [
        jax.device_put(
            np.concatenate([np.asarray(in_maps[c][n]) for c in range(n_cores)], 0), sh)
        for n in in_names
    ]
    concat_zero = [
        jax.device_put(np.zeros((n_cores * z.shape[0], *z.shape[1:]), z.dtype), sh)
        for z in zero_outs
    ]
    # warmup (compiles)
    r = f(*concat_in, *concat_zero)
    jax.block_until_ready(r)
    times = []
    for _ in range(iters):
        t0 = time.perf_counter()
        r = f(*concat_in, *concat_zero)
        jax.block_until_ready(r)
        times.append(time.perf_counter() - t0)
    return min(times), times


def kernel(word_input, input_mask, labels, labels_token, data_type, word_emb,
           Wih_f, Whh_f, bih_f, bhh_f, Wih_b, Whh_b, bih_b, bhh_b,
           W_tag, b_tag, trans):
    global LAST_RESULT
    word_input = np.asarray(word_input).astype(np.int64)
    input_mask = np.asarray(input_mask).astype(np.int32)
    labels = np.asarray(labels).astype(np.int64)
    word_emb = np.asarray(word_emb, dtype=np.float32)
    W_tag = np.asarray(W_tag, dtype=np.float32)
    b_tag = np.asarray(b_tag, dtype=np.float32)
    trans = np.asarray(trans, dtype=np.float32)

    x = word_emb[word_input] * input_mask[..., None].astype(np.float32)

    pk_f = _pack_weights(np.asarray(Wih_f, np.float32), np.asarray(Whh_f, np.float32),
                         np.asarray(bih_f, np.float32), np.asarray(bhh_f, np.float32),
                         W_tag[:, :H])
    pk_b = _pack_weights(np.asarray(Wih_b, np.float32), np.asarray(Whh_b, np.float32),
                         np.asarray(bih_b, np.float32), np.asarray(bhh_b, np.float32),
                         W_tag[:, H:])

    in_maps = []
    for q in range(4):
        xq = x[q * BL:(q + 1) * BL]
        in_maps.append({"xt": _pack_x(xq),
                        "wih": pk_f[0], "whh": pk_f[1], "wtg": pk_f[2]})
    for q in range(4):
        xq = x[q * BL:(q + 1) * BL][:, ::-1]
        in_maps.append({"xt": _pack_x(xq),
                        "wih": pk_b[0], "whh": pk_b[1], "wtg": pk_b[2]})

    global LAST_IN_MAPS
    LAST_IN_MAPS = in_maps
    nc = _get_nc()
    res = run_bass_kernel_spmd(nc, in_maps, core_ids=list(range(8)))
    LAST_RESULT = res

    logits = np.empty((B, S, T), np.float32)
    for q in range(4):
        lf = res.results[q]["p"].reshape(T, S, BL)
        lb = res.results[4 + q]["p"].reshape(T, S, BL)[:, ::-1, :]
        logits[q * BL:(q + 1) * BL] = (lf + lb).transpose(2, 1, 0) + b_tag

    crf = _crf_nll_np(logits, input_mask, labels, trans)
    ans_loss = np.float32(crf / B)
    tags = _viterbi_np(logits, input_mask, trans).astype(np.int32)
    return ans_loss, tags


# revision 17
# speedup vs baseline: 19.6106x; 19.6106x over previous
"""BiLSTM + CRF (NCRF++ style) Trainium2 Bass kernel.

Sharding: direction-split + batch-split across 8 NeuronCores.
  cores 0-3: forward LSTM, batch quarters 0-3 (8 seqs each)
  cores 4-7: backward LSTM (time-reversed input data), batch quarters 0-3
All cores run the SAME NEFF; only input data differs (SPMD).

Per-core device work:
  1. input projection xW = x @ Wih'.T + b'  (batched over all tokens)
  2. 256-step LSTM recurrence (weights-stationary fp32 matmuls,
     gate rows on the 128 partitions)
  3. partial logit projection P = h_all @ W_tag_half.T

Gate rows are host-reordered to [i,f,o,g] so one Sigmoid ACT op covers
i,f,o (contiguous free cols) and one Tanh covers g. The i/f/o/g biases
fold into the input projection via a constant-1 feature column.

Host does: embedding gather, data packing, CRF-NLL + Viterbi tail
(0.02% of FLOPs) in numpy.
"""

import numpy as np


def _ensure_path():
    import sys
    for p in ("/opt/trn_rl_repo",):
        if p not in sys.path:
            sys.path.insert(0, p)


_ensure_path()

try:
    from antenv import axon_hooks as _ah  # noqa: F401
except Exception:
    import sys as _s
    import types as _t
    _m = _t.ModuleType("antenv.axon_hooks")
    _m.get_axon_ntff_profile_hook = lambda: None
    _s.modules["antenv.axon_hooks"] = _m

import bass_rust  # noqa: E402
import concourse.bass as bass  # noqa: E402
import concourse.mybir as mybir  # noqa: E402
from concourse.tile import TileContext  # noqa: E402
from concourse.bass_utils import run_bass_kernel_spmd  # noqa: E402

B, S, E, H = 32, 256, 300, 256
T = 19
START, STOP = 17, 18
EP = 384           # E padded to 3*128
BL = B // 4        # sequences per core (8)
KC_X = EP // 128   # K chunks for input projection (3)
KC_H = H // 128    # K chunks for Whh (2)
MC = (4 * H) // 128  # M chunks of gate dim (8)
NTOK = S * BL      # tokens per core (2048)
NBLK = NTOK // 512  # token blocks for batched matmuls (4)

F32 = mybir.dt.float32
F32R = mybir.dt.float32r
BF16 = mybir.dt.bfloat16
ADD = mybir.AluOpType.add
MULT = mybir.AluOpType.mult
TANH = mybir.ActivationFunctionType.Tanh
SIGM = mybir.ActivationFunctionType.Sigmoid

LAST_RESULT = None
LAST_IN_MAPS = None
_NC_CACHE = {}


def _build_bass():
    nc = bass.Bass()
    xt_d = nc.dram_tensor("xt", [128, KC_X, NTOK], F32, kind="ExternalInput")
    wih_d = nc.dram_tensor("wih", [128, KC_X, MC, 128], F32, kind="ExternalInput")
    whh_d = nc.dram_tensor("whh", [128, KC_H, MC, 128], F32, kind="ExternalInput")
    wtg_d = nc.dram_tensor("wtg", [128, KC_H, T], F32, kind="ExternalInput")
    p_d = nc.dram_tensor("p", [T, NTOK], F32, kind="ExternalOutput")

    with TileContext(nc) as tc:
        with tc.tile_pool(name="const", bufs=1) as cpool, \
             tc.tile_pool(name="xw", bufs=1) as xwpool, \
             tc.tile_pool(name="hall", bufs=1) as hpool, \
             tc.tile_pool(name="work", bufs=3) as wpool, \
             tc.tile_pool(name="state", bufs=2) as spool, \
             tc.tile_pool(name="psx", bufs=4, space="PSUM") as psxpool, \
             tc.tile_pool(name="psg", bufs=3, space="PSUM") as psgpool, \
             tc.tile_pool(name="psl", bufs=1, space="PSUM") as pslpool:

            xt = cpool.tile([128, KC_X, NTOK], F32R)
            wih = cpool.tile([128, KC_X, MC, 128], F32R)
            whh = cpool.tile([128, KC_H, MC, 128], F32R)
            wtg = cpool.tile([128, KC_H, T], F32R)
            # Funnel all PE-read tiles through DVE copies: a self-loading
            # fp32 Matmult has very few semaphore-wait slots, so all its
            # upstream deps must collapse onto one (DVE) semaphore.
            with tc.tile_pool(name="stage", bufs=1) as stpool:
                xt_s = stpool.tile([128, KC_X, NTOK], F32)
                wih_s = stpool.tile([128, KC_X, MC, 128], F32)
                whh_s = stpool.tile([128, KC_H, MC, 128], F32)
                wtg_s = stpool.tile([128, KC_H, T], F32)
                nc.sync.dma_start(wih_s[:, :, :, :], wih_d[:, :, :, :])
                nc.sync.dma_start(whh_s[:, :, :, :], whh_d[:, :, :, :])
                nc.sync.dma_start(wtg_s[:, :, :], wtg_d[:, :, :])
                for k in range(KC_X):
                    nc.sync.dma_start(xt_s[:, k, :], xt_d[:, k, :])
                nc.vector.tensor_copy(wih[:, :, :, :], wih_s[:, :, :, :])
                nc.vector.tensor_copy(whh[:, :, :, :], whh_s[:, :, :, :])
                nc.vector.tensor_copy(wtg[:, :, :], wtg_s[:, :, :])
                for k in range(KC_X):
                    nc.vector.tensor_copy(xt[:, k, :], xt_s[:, k, :])

            # xW[p, t, m, b]: pre-gate input contribution per token
            xw = xwpool.tile([128, S, MC, BL], F32)
            # h_all[p, k, t, b]: hidden states, rounded to f32r for the PE
            hall = hpool.tile([128, KC_H, S, BL], F32R)

            # ---- phase 1: input projection ----
            for nb in range(NBLK):
                for m in range(MC):
                    ps = psxpool.tile([128, 64, BL], F32)
                    for k in range(KC_X):
                        nc.tensor.matmul(
                            ps[:, :, :],
                            wih[:, k, m, :],
                            xt[:, k, nb * 512:(nb + 1) * 512],
                            start=(k == 0), stop=(k == KC_X - 1),
                        )
                    nc.vector.tensor_copy(
                        xw[:, nb * 64:(nb + 1) * 64, m, :], ps[:, :, :])

            # ---- phase 2: LSTM recurrence ----
            c_prev = spool.tile([128, KC_H, BL], F32, tag="c")
            nc.vector.memset(c_prev[:, :, :], 0.0)
            for t in range(S):
                if t > 0:
                    g = psgpool.tile([128, MC, BL], F32)
                    for m in range(MC):
                        for k in range(KC_H):
                            nc.tensor.matmul(
                                g[:, m, :],
                                whh[:, k, m, :],
                                hall[:, k, t - 1, :],
                                start=(k == 0), stop=(k == KC_H - 1),
                            )
                    pre = wpool.tile([128, MC, BL], F32, tag="pre")
                    nc.vector.tensor_tensor(
                        pre[:, :, :], g[:, :, :], xw[:, t, :, :], ADD)
                    pre_ap = pre[:, :, :]
                else:
                    pre_ap = xw[:, 0, :, :]
                # gate chunks (host reorder): m 0-1=i, 2-3=f, 4-5=o, 6-7=g
                sg = wpool.tile([128, 6, BL], F32, tag="sg")
                nc.scalar.activation(sg[:, :, :], pre_ap[:, 0:6, :], SIGM)
                tg = wpool.tile([128, KC_H, BL], F32, tag="tg")
                nc.scalar.activation(tg[:, :, :], pre_ap[:, 6:8, :], TANH)
                t1 = wpool.tile([128, KC_H, BL], F32, tag="t1")
                t2 = wpool.tile([128, KC_H, BL], F32, tag="t2")
                c_new = spool.tile([128, KC_H, BL], F32, tag="c")
                nc.vector.tensor_tensor(
                    t1[:, :, :], sg[:, 2:4, :], c_prev[:, :, :], MULT)
                nc.vector.tensor_tensor(
                    t2[:, :, :], sg[:, 0:2, :], tg[:, :, :], MULT)
                nc.vector.tensor_tensor(
                    c_new[:, :, :], t1[:, :, :], t2[:, :, :], ADD)
                tct = wpool.tile([128, KC_H, BL], F32, tag="tct")
                nc.scalar.activation(tct[:, :, :], c_new[:, :, :], TANH)
                nc.vector.tensor_tensor(
                    hall[:, :, t, :], sg[:, 4:6, :], tct[:, :, :], MULT)
                c_prev = c_new

            # ---- phase 3: partial logits ----
            p_sb = cpool.tile([T, NTOK], F32)
            for nb in range(NBLK):
                pl = pslpool.tile([T, 512], F32)
                for k in range(KC_H):
                    nc.tensor.matmul(
                        pl[:, :],
                        wtg[:, k, :],
                        hall[:, k, nb * 64:(nb + 1) * 64, :],
                        start=(k == 0), stop=(k == KC_H - 1),
                    )
                nc.vector.tensor_copy(p_sb[:, nb * 512:(nb + 1) * 512], pl[:, :])
            nc.sync.dma_start(p_d[:, :], p_sb[:, :])
    _split_multiwaits(nc)
    return nc


def _split_multiwaits(nc):
    """This toolchain's walrus accepts only ONE sync-wait per compute
    instruction. Split extra waits onto engine NoOps inserted just
    before the offender (same engine, same position => semantics kept)."""
    wid = 0
    for f in nc.m.functions:
        for bb in f.blocks:
            out = []
            changed = False
            for inst in bb.instructions:
                si = inst.sync_info
                if si is not None and si.on_wait is not None and len(si.on_wait) > 1:
                    waits = list(si.on_wait)
                    for w in waits[:-1]:
                        wid += 1
                        out.append(bass_rust.InstNoOp(
                            name=f"WS-{wid}", engine=inst.engine,
                            sync_info=mybir.SyncInfo(on_wait=[w], on_update=[])))
                    inst.sync_info = mybir.SyncInfo(
                        on_wait=[waits[-1]],
                        on_update=list(si.on_update or []))
                    changed = True
                out.append(inst)
            if changed:
                bb.instructions = out


def _get_nc():
    if "nc" not in _NC_CACHE:
        _NC_CACHE["nc"] = _build_bass()
    return _NC_CACHE["nc"]


def _pack_weights(Wih, Whh, bih, bhh, Wtag_half):
    """Reorder gate rows i,f,g,o -> i,f,o,g and pack matmul layouts."""
    perm = np.concatenate(
        [np.arange(0, 512), np.arange(768, 1024), np.arange(512, 768)])
    Wih2 = Wih[perm].astype(np.float32)
    Whh2 = Whh[perm].astype(np.float32)
    b2 = (bih + bhh)[perm].astype(np.float32)

    Wp = np.zeros((4 * H, EP), np.float32)
    Wp[:, :E] = Wih2
    Wp[:, E] = b2  # bias folded in via constant-1 feature row E
    wih_arr = np.ascontiguousarray(
        Wp.T.reshape(KC_X, 128, MC, 128).transpose(1, 0, 2, 3))
    whh_arr = np.ascontiguousarray(
        Whh2.T.reshape(KC_H, 128, MC, 128).transpose(1, 0, 2, 3))
    Wt = Wtag_half.astype(np.float32)  # [T, H]
    wtg_arr = np.ascontiguousarray(
        Wt.T.reshape(KC_H, 128, T).transpose(1, 0, 2))
    return wih_arr, whh_arr, wtg_arr


def _pack_x(xq):
    """xq [BL, S, E] -> [128, KC_X, NTOK] with token j = t*BL + b."""
    xp = np.zeros((BL, S, EP), np.float32)
    xp[:, :, :E] = xq
    xp[:, :, E] = 1.0
    arr = xp.transpose(2, 1, 0).reshape(KC_X, 128, NTOK)
    return np.ascontiguousarray(arr.transpose(1, 0, 2))


def _crf_nll_np(emit, mask, labels, trans):
    emit = emit.astype(np.float64)
    trans = trans.astype(np.float64)
    mf = mask.astype(np.float64)
    b = emit.shape[0]
    prev = np.concatenate(
        [np.full((b, 1), START, labels.dtype), labels[:, :-1]], axis=1)
    trans_sc = trans[prev, labels]
    emit_sc = np.take_along_axis(emit, labels[..., None], axis=-1)[..., 0]
    gold = np.sum((trans_sc + emit_sc) * mf, axis=1)
    lengths = mask.sum(axis=1).astype(np.int64)
    last_tag = np.take_along_axis(labels, (lengths - 1)[:, None], axis=1)[:, 0]
    gold = gold + trans[last_tag, STOP]
    alpha = emit[:, 0] + trans[START][None, :]
    for t in range(1, emit.shape[1]):
        cand = alpha[:, :, None] + trans[None]        # [B, prev, cur]
        mx = cand.max(axis=1)
        new = mx + np.log(np.exp(cand - mx[:, None, :]).sum(axis=1)) + emit[:, t]
        alpha = np.where(mask[:, t][:, None] > 0, new, alpha)
    final = alpha + trans[:, STOP][None, :]
    mx = final.max(axis=1)
    logZ = mx + np.log(np.exp(final - mx[:, None]).sum(axis=1))
    return np.sum(logZ - gold)


def _viterbi_np(emit, mask, trans):
    emit = emit.astype(np.float32)
    b, s, t = emit.shape
    delta = emit[:, 0] + trans[START][None, :]
    idT = np.arange(t, dtype=np.int32)[None, :]
    ptrs = []
    for i in range(1, s):
        cand = delta[:, :, None] + trans[None]        # [B, prev, cur]
        best_prev = cand.argmax(axis=1).astype(np.int32)
        new = cand.max(axis=1) + emit[:, i]
        m = mask[:, i][:, None] > 0
        delta = np.where(m, new, delta)
        ptrs.append(np.where(m, best_prev, idT))
    final = delta + trans[:, STOP][None, :]
    best_last = final.argmax(axis=1).astype(np.int32)
    bidx = np.arange(b)
    ys = np.empty((b, s - 1), np.int32)
    carry = best_last
    for i in range(s - 2, -1, -1):
        ys[:, i] = carry
        carry = ptrs[i][bidx, carry]
    tags = np.concatenate([carry[:, None], ys], axis=1)
    return tags * (mask > 0)


def bench_device(in_maps, iters=10, inner=1):
    """Time the NEFF execution via PJRT with device-resident inputs.

    Mirrors bass2jax.run_bass_via_pjrt's shard_map path but without
    donation, inputs device_put once, executable reused across iters.
    Returns per-iteration wall seconds (min over iters), which bounds
    the HW exec time from above (adds one dispatch round-trip).
    """
    import time
    import jax
    from jax.experimental.shard_map import shard_map
    from jax.sharding import Mesh, NamedSharding, PartitionSpec
    from concourse import bass2jax as b2j

    nc = _get_nc()
    b2j.install_neuronx_cc_hook()
    n_cores = len(in_maps)
    partition_name = (nc.partition_id_tensor.name
                      if nc.partition_id_tensor else None)

    in_names, out_names, out_avals, zero_outs = [], [], [], []
    for alloc in nc.m.functions[0].allocations:
        if not isinstance(alloc, mybir.MemoryLocationSet):
            continue
        name = alloc.memorylocations[0].name
        if alloc.kind == "ExternalInput":
            if name != partition_name:
                in_names.append(name)
        elif alloc.kind == "ExternalOutput":
            shape = tuple(alloc.tensor_shape)
            dtype = mybir.dt.np(alloc.dtype)
            out_names.append(name)
            out_avals.append(jax.core.ShapedArray(shape, dtype))
            zero_outs.append(np.zeros(shape, dtype))
    n_params = len(in_names)
    all_names = in_names + out_names
    if partition_name is not None:
        all_names = all_names + [partition_name]

    import jax as _jax

    def _exec_once(operands):
        outs = b2j._bass_exec_p.bind(
            *operands,
            out_avals=tuple(out_avals),
            in_names=tuple(all_names),
            out_names=tuple(out_names),
            lowering_input_output_aliases=(),
            sim_require_finite=True,
            sim_require_nnan=True,
            nc=nc,
        )
        return tuple(outs)

    def _body(*args):
        operands = list(args)
        if partition_name is not None:
            operands.append(b2j.partition_id_tensor())
        if inner <= 1:
            return _exec_once(operands)

        def step(carry, _):
            outs = _exec_once(operands)
            return outs, None
        init = tuple(_jax.numpy.zeros(a.shape, a.dtype) for a in out_avals)
        final, _ = _jax.lax.scan(step, init, None, length=inner)
        return final

    devices = jax.devices()[:n_cores]
    mesh = Mesh(np.asarray(devices), ("core",))
    spec = PartitionSpec("core")
    in_specs = (spec,) * (n_params + len(out_names))
    out_specs = (spec,) * len(out_names)
    f = jax.jit(shard_map(_body, mesh=mesh, in_specs=in_specs,
                          out_specs=out_specs, check_rep=False),
                keep_unused=True)
    sh = NamedSharding(mesh, spec)
    concat_in = [
        jax.device_put(
            np.concatenate([np.asarray(in_maps[c][n]) for c in range(n_cores)], 0), sh)
        for n in in_names
    ]
    concat_zero = [
        jax.device_put(np.zeros((n_cores * z.shape[0], *z.shape[1:]), z.dtype), sh)
        for z in zero_outs
    ]
    # warmup (compiles)
    r = f(*concat_in, *concat_zero)
    jax.block_until_ready(r)
    times = []
    for _ in range(iters):
        t0 = time.perf_counter()
        r = f(*concat_in, *concat_zero)
        jax.block_until_ready(r)
        times.append(time.perf_counter() - t0)
    return min(times), times


def kernel(word_input, input_mask, labels, labels_token, data_type, word_emb,
           Wih_f, Whh_f, bih_f, bhh_f, Wih_b, Whh_b, bih_b, bhh_b,
           W_tag, b_tag, trans):
    global LAST_RESULT
    word_input = np.asarray(word_input).astype(np.int64)
    input_mask = np.asarray(input_mask).astype(np.int32)
    labels = np.asarray(labels).astype(np.int64)
    word_emb = np.asarray(word_emb, dtype=np.float32)
    W_tag = np.asarray(W_tag, dtype=np.float32)
    b_tag = np.asarray(b_tag, dtype=np.float32)
    trans = np.asarray(trans, dtype=np.float32)

    x = word_emb[word_input] * input_mask[..., None].astype(np.float32)

    pk_f = _pack_weights(np.asarray(Wih_f, np.float32), np.asarray(Whh_f, np.float32),
                         np.asarray(bih_f, np.float32), np.asarray(bhh_f, np.float32),
                         W_tag[:, :H])
    pk_b = _pack_weights(np.asarray(Wih_b, np.float32), np.asarray(Whh_b, np.float32),
                         np.asarray(bih_b, np.float32), np.asarray(bhh_b, np.float32),
                         W_tag[:, H:])

    in_maps = []
    for q in range(4):
        xq = x[q * BL:(q + 1) * BL]
        in_maps.append({"xt": _pack_x(xq),
                        "wih": pk_f[0], "whh": pk_f[1], "wtg": pk_f[2]})
    for q in range(4):
        xq = x[q * BL:(q + 1) * BL][:, ::-1]
        in_maps.append({"xt": _pack_x(xq),
                        "wih": pk_b[0], "whh": pk_b[1], "wtg": pk_b[2]})

    global LAST_IN_MAPS
    LAST_IN_MAPS = in_maps
    nc = _get_nc()
    res = run_bass_kernel_spmd(nc, in_maps, core_ids=list(range(8)))
    LAST_RESULT = res

    logits = np.empty((B, S, T), np.float32)
    for q in range(4):
        lf = res.results[q]["p"].reshape(T, S, BL)
        lb = res.results[4 + q]["p"].reshape(T, S, BL)[:, ::-1, :]
        logits[q * BL:(q + 1) * BL] = (lf + lb).transpose(2, 1, 0) + b_tag

    crf = _crf_nll_np(logits, input_mask, labels, trans)
    ans_loss = np.float32(crf / B)
    tags = _viterbi_np(logits, input_mask, trans).astype(np.int32)
    return ans_loss, tags
